# revision 1
# baseline (speedup 1.0000x reference)
"""DCNv2 (modulated deformable conv) forward on 8 Trainium2 NeuronCores.

Strategy: data-parallel over batch (B=8, one batch per core).  Per core:
  1. PE GEMM (bf16): z_k^T[j, oc] = x^T[j, :] @ W_k for the 9 kernel taps
     (x-tile stationary, W moving), stored as a spatial-major table in DRAM
     (rows of 256 oc, 512 B).
  2. SWDGE dma_gather: bilinear corners fetched as row PAIRS (r, r+1) — one
     descriptor covers the (x0, x0+1) corner pair of a tap; two descriptors
     (y0-row, y1-row) cover all 4 corners.
  3. DVE scalar_tensor_tensor: acc[j, oc] += w_slot[j] * G_slot[j, oc]
     (mask + bilinear weights + border validity folded into w_slot on host;
     bias folded into the first term).
  4. PE transpose (j, oc) -> (oc, j) and DMA out.

Index/weight prep (floor, fractional weights, border handling) runs on host
in numpy — it is O(B*K*H*W) marshalling, ~0.003% of the conv FLOPs.
"""

from contextlib import ExitStack

import ml_dtypes
import numpy as np

import concourse.bass as bass
import concourse.bacc as bacc
import concourse.mybir as mybir
import concourse.tile as tile
from concourse.bass_utils import run_bass_kernel_spmd
from concourse.masks import make_identity

F32 = mybir.dt.float32
BF16 = mybir.dt.bfloat16
I16 = mybir.dt.int16

# problem constants (hardcoded per harness contract)
B = 8
C = 256
OC = 256
H = W = 64
HW = H * W
K = 9
KH = KW = 3
PAD = 1

T = K
JT = 32            # j-tiles of 128 output positions
CT = 2             # 128-channel contraction tiles
JPG = 16           # j-tiles per dma_gather call
HALVES = JT // JPG
NIDX = JPG * 128
ZROWS = HW + 1
USE_BF16_ACC = True
NQUEUES = 4

_program_cache = {}


def build_program():
    if "nc" in _program_cache:
        return _program_cache["nc"]
    nc = bacc.Bacc("TRN2", target_bir_lowering=False, debug=False,
                   num_swdge_queues=NQUEUES)

    x_d = nc.dram_tensor("x", [128, CT, HW], BF16, kind="ExternalInput")
    wr_d = nc.dram_tensor("wr", [128, T, CT, OC], BF16, kind="ExternalInput")
    idx_d = nc.dram_tensor("idx", [128, T, HALVES, 2, NIDX // 16], I16,
                           kind="ExternalInput")
    w4_d = nc.dram_tensor("w4", [128, T, 4, JT], BF16, kind="ExternalInput")
    bias_d = nc.dram_tensor("biasb", [128, OC], F32, kind="ExternalInput")
    out_d = nc.dram_tensor("out", [2, JT, 128, 128], F32, kind="ExternalOutput")

    with tile.TileContext(nc) as tc, ExitStack() as ctx:
        sp = ctx.enter_context(tc.tile_pool(name="sbuf", bufs=1))
        zst_p = ctx.enter_context(tc.tile_pool(name="zst", bufs=6))
        g_p = ctx.enter_context(tc.tile_pool(name="g", bufs=4))
        acc_p = ctx.enter_context(tc.tile_pool(name="acc", bufs=JT))
        ost_p = ctx.enter_context(tc.tile_pool(name="ost", bufs=4))
        ps_p = ctx.enter_context(tc.tile_pool(name="psum", bufs=6, space="PSUM"))
        pst_p = ctx.enter_context(tc.tile_pool(name="psumt", bufs=2, space="PSUM"))
        z_p = ctx.enter_context(tc.tile_pool(name="zdram", bufs=T, space="DRAM"))

        x_sb = sp.tile([128, CT, HW], BF16)
        nc.sync.dma_start(x_sb[:], x_d.ap())
        wr_sb = sp.tile([128, T, CT, OC], BF16)
        nc.sync.dma_start(wr_sb[:], wr_d.ap())
        idx_sb = sp.tile([128, T, HALVES, 2, NIDX // 16], I16)
        nc.sync.dma_start(idx_sb[:], idx_d.ap())
        w4_sb = sp.tile([128, T, 4, JT], BF16)
        nc.sync.dma_start(w4_sb[:], w4_d.ap())
        ACC_DT = BF16 if USE_BF16_ACC else F32
        bias_sb = sp.tile([128, OC], ACC_DT)
        nc.gpsimd.dma_start(bias_sb[:], bias_d.ap())
        ident = sp.tile([128, 128], ACC_DT)
        make_identity(nc, ident[:])

        zero_row = sp.tile([1, OC], BF16)
        nc.vector.memset(zero_row[:], 0.0)

        z_tiles = []
        for k in range(T):
            zk = z_p.tile([ZROWS, OC], BF16, name=f"z{k}", tag="z")
            z_tiles.append(zk)
            # slot1 of row HW-1 reads row HW: its weight is always 0, but the
            # value must be finite (0 * NaN = NaN), so zero it.
            nc.sync.dma_start(zk[HW:HW + 1, :], zero_row[:])

        acc = [acc_p.tile([128, OC], ACC_DT, name=f"acc{j}", tag="acc")
               for j in range(JT)]
        acc_init = [False] * JT

        # phase 1 (tap-major so tap k's gathers overlap tap k+1's GEMM)
        for k in range(T):
            for jt in range(JT):
                ps = ps_p.tile([128, OC], F32)
                for ct in range(CT):
                    nc.tensor.matmul(
                        ps[:],
                        x_sb[:, ct, jt * 128:(jt + 1) * 128],
                        wr_sb[:, k, ct, :],
                        start=(ct == 0),
                        stop=(ct == CT - 1),
                    )
                zst = zst_p.tile([128, OC], BF16)
                nc.scalar.copy(zst[:], ps[:])
                nc.sync.dma_start(z_tiles[k][jt * 128:(jt + 1) * 128, :], zst[:])

            # phases 2+3 for this tap
            zk_ap = z_tiles[k][:]
            win_ap = bass.AP(zk_ap.tensor, zk_ap.offset, [[OC, HW], [1, 2 * OC]])
            for half in range(HALVES):
                g01 = []
                for rsel in range(2):
                    g = g_p.tile([128, JPG, 2 * OC], BF16)
                    nc.gpsimd.dma_gather(
                        out_ap=g[:],
                        in_ap=win_ap,
                        idxs_ap=idx_sb[:, k, half, rsel, :],
                        num_idxs=NIDX,
                        num_idxs_reg=NIDX,
                        elem_size=2 * OC,
                        elem_step=OC,
                        single_packet=False,
                        queue_num=(k * HALVES * 2 + half * 2 + rsel) % NQUEUES,
                    )
                    g01.append(g)
                for gi in range(JPG):
                    jt = half * JPG + gi
                    for rsel in range(2):
                        for sub in range(2):
                            slot = rsel * 2 + sub
                            in1 = acc[jt][:] if acc_init[jt] else bias_sb[:]
                            acc_init[jt] = True
                            nc.vector.scalar_tensor_tensor(
                                out=acc[jt][:],
                                in0=g01[rsel][:, gi, sub * OC:(sub + 1) * OC],
                                scalar=w4_sb[:, k, slot, jt:jt + 1],
                                in1=in1,
                                op0=mybir.AluOpType.mult,
                                op1=mybir.AluOpType.add,
                            )

        # phase 4
        for jt in range(JT):
            for och in range(2):
                pt = pst_p.tile([128, 128], ACC_DT)
                nc.tensor.transpose(
                    pt[:], acc[jt][:, och * 128:(och + 1) * 128], ident[:]
                )
                ost = ost_p.tile([128, 128], F32)
                nc.scalar.copy(ost[:], pt[:])
                nc.sync.dma_start(out_d.ap()[och, jt], ost[:])

    nc.compile()
    _program_cache["nc"] = nc
    return nc


def _prep_indices_weights(offset_b, mask_b):
    """Per-batch (18,64,64)/(9,64,64) f32 -> pair-base rows r0,r1 (9,4096) and
    slot weights w4 (9,4,4096) with bilinear/mask/validity folded in."""
    off = offset_b.reshape(K, 2, H, W).astype(np.float32)
    m = mask_b.reshape(K, H, W).astype(np.float32)

    oy = np.arange(H, dtype=np.float32) - PAD
    ox = np.arange(W, dtype=np.float32) - PAD
    ky = np.repeat(np.arange(KH, dtype=np.float32), KW)
    kx = np.tile(np.arange(KW, dtype=np.float32), KH)

    py = ky[:, None, None] + oy[None, :, None] + off[:, 0]
    px = kx[:, None, None] + ox[None, None, :] + off[:, 1]

    y0 = np.floor(py)
    x0 = np.floor(px)
    wy = py - y0
    wx = px - x0
    y0i = y0.astype(np.int64)
    x0i = x0.astype(np.int64)

    vy0 = (y0i >= 0) & (y0i < H)
    vy1 = (y0i + 1 >= 0) & (y0i + 1 < H)
    vx0 = (x0i >= 0) & (x0i < W)
    vx1 = (x0i + 1 >= 0) & (x0i + 1 < W)

    w00 = (1 - wy) * (1 - wx) * vy0 * vx0 * m
    w01 = (1 - wy) * wx * vy0 * vx1 * m
    w10 = wy * (1 - wx) * vy1 * vx0 * m
    w11 = wy * wx * vy1 * vx1 * m

    neg_x = x0i < 0
    bx = np.clip(x0i, 0, W - 1)
    s00 = np.where(neg_x, w01, w00)  # x0 == -1: the x0+1 corner sits at slot0
    s01 = np.where(neg_x, 0.0, w01)
    s10 = np.where(neg_x, w11, w10)
    s11 = np.where(neg_x, 0.0, w11)

    yc0 = np.clip(y0i, 0, H - 1)
    yc1 = np.clip(y0i + 1, 0, H - 1)
    r0 = (yc0 * W + bx).reshape(K, HW).astype(np.int32)
    r1 = (yc1 * W + bx).reshape(K, HW).astype(np.int32)

    w4 = np.stack(
        [s00.reshape(K, HW), s01.reshape(K, HW),
         s10.reshape(K, HW), s11.reshape(K, HW)], axis=1
    ).astype(np.float32)
    return r0, r1, w4


def _prep_core_inputs(x_b, offset_b, mask_b, weight, bias):
    r0, r1, w4 = _prep_indices_weights(offset_b, mask_b)

    x_in = np.ascontiguousarray(
        x_b.reshape(CT, 128, HW).transpose(1, 0, 2)
    ).astype(ml_dtypes.bfloat16)

    wk = weight.reshape(OC, CT, 128, K)
    wr = np.ascontiguousarray(wk.transpose(2, 3, 1, 0)).astype(ml_dtypes.bfloat16)

    rs = np.stack([r0, r1], axis=1).reshape(K, 2, HALVES, NIDX)
    rs = rs.transpose(0, 2, 1, 3)  # [k, half, rsel, i]
    s_idx = np.arange(NIDX // 16)
    p_idx = np.arange(128)
    wrapped = rs[:, :, :, (s_idx[None, :] * 16 + (p_idx[:, None] % 16))]
    idx_in = np.ascontiguousarray(wrapped.transpose(3, 0, 1, 2, 4)).astype(np.int16)

    w4r = w4.reshape(K, 4, JT, 128)
    w4_in = np.ascontiguousarray(w4r.transpose(3, 0, 1, 2)).astype(ml_dtypes.bfloat16)

    bias_in = np.ascontiguousarray(
        np.broadcast_to(bias[None, :], (128, OC))
    ).astype(np.float32)

    return {"x": x_in, "wr": wr, "idx": idx_in, "w4": w4_in, "biasb": bias_in}


def kernel(x, offset, mask, weight, bias):
    x = np.asarray(x, dtype=np.float32)
    offset = np.asarray(offset, dtype=np.float32)
    mask = np.asarray(mask, dtype=np.float32)
    weight = np.asarray(weight, dtype=np.float32)
    bias = np.asarray(bias, dtype=np.float32)

    nc = build_program()
    in_maps = [
        _prep_core_inputs(x[b], offset[b], mask[b], weight, bias)
        for b in range(B)
    ]
    res = run_bass_kernel_spmd(nc, in_maps, core_ids=list(range(B)))

    out = np.empty((B, OC, H, W), dtype=np.float32)
    for b in range(B):
        o = res.results[b]["out"]  # (2, JT, 128, 128)
        out[b] = o.transpose(0, 2, 1, 3).reshape(OC, H, W)
    return out



# revision 3
# speedup vs baseline: 1.1377x; 1.1377x over previous
"""DCNv2 (modulated deformable conv) forward on 8 Trainium2 NeuronCores.

Strategy: data-parallel over batch (B=8, one batch per core).  Per core:
  1. PE GEMM (bf16): z_k^T[j, oc] = x^T[j, :] @ W_k for the 9 kernel taps
     (x-tile stationary, W moving), stored as a spatial-major table in DRAM
     (rows of 256 oc, 512 B).
  2. SWDGE dma_gather: bilinear corners fetched as row PAIRS (r, r+1) — one
     descriptor covers the (x0, x0+1) corner pair of a tap; two descriptors
     (y0-row, y1-row) cover all 4 corners.
  3. DVE scalar_tensor_tensor: acc[j, oc] += w_slot[j] * G_slot[j, oc]
     (mask + bilinear weights + border validity folded into w_slot on host;
     bias folded into the first term).
  4. PE transpose (j, oc) -> (oc, j) and DMA out.

Index/weight prep (floor, fractional weights, border handling) runs on host
in numpy — it is O(B*K*H*W) marshalling, ~0.003% of the conv FLOPs.
"""

from contextlib import ExitStack

import ml_dtypes
import numpy as np

import concourse.bass as bass
import concourse.bacc as bacc
import concourse.mybir as mybir
import concourse.tile as tile
from concourse.bass_utils import run_bass_kernel_spmd
from concourse.masks import make_identity

F32 = mybir.dt.float32
BF16 = mybir.dt.bfloat16
I16 = mybir.dt.int16

# problem constants (hardcoded per harness contract)
B = 8
C = 256
OC = 256
H = W = 64
HW = H * W
K = 9
KH = KW = 3
PAD = 1

T = K
JT = 32            # j-tiles of 128 output positions
CT = 2             # 128-channel contraction tiles
JPG = 8            # j-tiles per dma_gather call
HALVES = JT // JPG
NIDX = JPG * 128
ZROWS = HW + 1
USE_BF16_ACC = True
NQUEUES = 4

_program_cache = {}


def build_program():
    if "nc" in _program_cache:
        return _program_cache["nc"]
    nc = bacc.Bacc("TRN2", target_bir_lowering=False, debug=False,
                   num_swdge_queues=NQUEUES,
                   dynamic_dma_scratch_size=32768)

    x_d = nc.dram_tensor("x", [128, CT, HW], BF16, kind="ExternalInput")
    wr_d = nc.dram_tensor("wr", [128, T, CT, OC], BF16, kind="ExternalInput")
    idx_d = nc.dram_tensor("idx", [128, T, HALVES, 2, NIDX // 16], I16,
                           kind="ExternalInput")
    w4_d = nc.dram_tensor("w4", [128, T, 4, JT], BF16, kind="ExternalInput")
    bias_d = nc.dram_tensor("biasb", [128, OC], F32, kind="ExternalInput")
    out_d = nc.dram_tensor("out", [2, JT, 128, 128], F32, kind="ExternalOutput")

    with tile.TileContext(nc) as tc, ExitStack() as ctx:
        sp = ctx.enter_context(tc.tile_pool(name="sbuf", bufs=1))
        zst_p = ctx.enter_context(tc.tile_pool(name="zst", bufs=6))
        g_p = ctx.enter_context(tc.tile_pool(name="g", bufs=4))
        acc_p = ctx.enter_context(tc.tile_pool(name="acc", bufs=JT))
        ost_p = ctx.enter_context(tc.tile_pool(name="ost", bufs=4))
        ps_p = ctx.enter_context(tc.tile_pool(name="psum", bufs=6, space="PSUM"))
        pst_p = ctx.enter_context(tc.tile_pool(name="psumt", bufs=2, space="PSUM"))
        z_p = ctx.enter_context(tc.tile_pool(name="zdram", bufs=T, space="DRAM"))

        x_sb = sp.tile([128, CT, HW], BF16)
        nc.sync.dma_start(x_sb[:], x_d.ap())
        wr_sb = sp.tile([128, T, CT, OC], BF16)
        nc.sync.dma_start(wr_sb[:], wr_d.ap())
        idx_sb = sp.tile([128, T, HALVES, 2, NIDX // 16], I16)
        nc.sync.dma_start(idx_sb[:], idx_d.ap())
        w4_sb = sp.tile([128, T, 4, JT], BF16)
        nc.sync.dma_start(w4_sb[:], w4_d.ap())
        ACC_DT = BF16 if USE_BF16_ACC else F32
        bias_sb = sp.tile([128, OC], ACC_DT)
        nc.gpsimd.dma_start(bias_sb[:], bias_d.ap())
        ident = sp.tile([128, 128], ACC_DT)
        make_identity(nc, ident[:])

        zero_row = sp.tile([1, OC], BF16)
        nc.vector.memset(zero_row[:], 0.0)

        z_tiles = []
        for k in range(T):
            zk = z_p.tile([ZROWS, OC], BF16, name=f"z{k}", tag="z")
            z_tiles.append(zk)
            # slot1 of row HW-1 reads row HW: its weight is always 0, but the
            # value must be finite (0 * NaN = NaN), so zero it.
            nc.sync.dma_start(zk[HW:HW + 1, :], zero_row[:])

        acc = [acc_p.tile([128, OC], ACC_DT, name=f"acc{j}", tag="acc")
               for j in range(JT)]
        acc_init = [False] * JT

        # phase 1 (tap-major so tap k's gathers overlap tap k+1's GEMM)
        for k in range(T):
            for jt in range(JT):
                ps = ps_p.tile([128, OC], F32)
                for ct in range(CT):
                    nc.tensor.matmul(
                        ps[:],
                        x_sb[:, ct, jt * 128:(jt + 1) * 128],
                        wr_sb[:, k, ct, :],
                        start=(ct == 0),
                        stop=(ct == CT - 1),
                    )
                zst = zst_p.tile([128, OC], BF16)
                nc.scalar.copy(zst[:], ps[:])
                nc.sync.dma_start(z_tiles[k][jt * 128:(jt + 1) * 128, :], zst[:])

            # phases 2+3 for this tap
            zk_ap = z_tiles[k][:]
            win_ap = bass.AP(zk_ap.tensor, zk_ap.offset, [[OC, HW], [1, 2 * OC]])
            for half in range(HALVES):
                g01 = []
                for rsel in range(2):
                    g = g_p.tile([128, JPG, 2 * OC], BF16)
                    nc.gpsimd.dma_gather(
                        out_ap=g[:],
                        in_ap=win_ap,
                        idxs_ap=idx_sb[:, k, half, rsel, :],
                        num_idxs=NIDX,
                        num_idxs_reg=NIDX,
                        elem_size=2 * OC,
                        elem_step=OC,
                        single_packet=False,
                        queue_num=(k * HALVES * 2 + half * 2 + rsel) % NQUEUES,
                    )
                    g01.append(g)
                for gi in range(JPG):
                    jt = half * JPG + gi
                    for rsel in range(2):
                        for sub in range(2):
                            slot = rsel * 2 + sub
                            in1 = acc[jt][:] if acc_init[jt] else bias_sb[:]
                            acc_init[jt] = True
                            nc.vector.scalar_tensor_tensor(
                                out=acc[jt][:],
                                in0=g01[rsel][:, gi, sub * OC:(sub + 1) * OC],
                                scalar=w4_sb[:, k, slot, jt:jt + 1],
                                in1=in1,
                                op0=mybir.AluOpType.mult,
                                op1=mybir.AluOpType.add,
                            )

        # phase 4
        for jt in range(JT):
            for och in range(2):
                pt = pst_p.tile([128, 128], ACC_DT)
                nc.tensor.transpose(
                    pt[:], acc[jt][:, och * 128:(och + 1) * 128], ident[:]
                )
                ost = ost_p.tile([128, 128], F32)
                nc.scalar.copy(ost[:], pt[:])
                nc.sync.dma_start(out_d.ap()[och, jt], ost[:])

    nc.compile()
    _program_cache["nc"] = nc
    return nc


def _prep_indices_weights(offset_b, mask_b):
    """Per-batch (18,64,64)/(9,64,64) f32 -> pair-base rows r0,r1 (9,4096) and
    slot weights w4 (9,4,4096) with bilinear/mask/validity folded in."""
    off = offset_b.reshape(K, 2, H, W).astype(np.float32)
    m = mask_b.reshape(K, H, W).astype(np.float32)

    oy = np.arange(H, dtype=np.float32) - PAD
    ox = np.arange(W, dtype=np.float32) - PAD
    ky = np.repeat(np.arange(KH, dtype=np.float32), KW)
    kx = np.tile(np.arange(KW, dtype=np.float32), KH)

    py = ky[:, None, None] + oy[None, :, None] + off[:, 0]
    px = kx[:, None, None] + ox[None, None, :] + off[:, 1]

    y0 = np.floor(py)
    x0 = np.floor(px)
    wy = py - y0
    wx = px - x0
    y0i = y0.astype(np.int64)
    x0i = x0.astype(np.int64)

    vy0 = (y0i >= 0) & (y0i < H)
    vy1 = (y0i + 1 >= 0) & (y0i + 1 < H)
    vx0 = (x0i >= 0) & (x0i < W)
    vx1 = (x0i + 1 >= 0) & (x0i + 1 < W)

    w00 = (1 - wy) * (1 - wx) * vy0 * vx0 * m
    w01 = (1 - wy) * wx * vy0 * vx1 * m
    w10 = wy * (1 - wx) * vy1 * vx0 * m
    w11 = wy * wx * vy1 * vx1 * m

    neg_x = x0i < 0
    bx = np.clip(x0i, 0, W - 1)
    s00 = np.where(neg_x, w01, w00)  # x0 == -1: the x0+1 corner sits at slot0
    s01 = np.where(neg_x, 0.0, w01)
    s10 = np.where(neg_x, w11, w10)
    s11 = np.where(neg_x, 0.0, w11)

    yc0 = np.clip(y0i, 0, H - 1)
    yc1 = np.clip(y0i + 1, 0, H - 1)
    r0 = (yc0 * W + bx).reshape(K, HW).astype(np.int32)
    r1 = (yc1 * W + bx).reshape(K, HW).astype(np.int32)

    w4 = np.stack(
        [s00.reshape(K, HW), s01.reshape(K, HW),
         s10.reshape(K, HW), s11.reshape(K, HW)], axis=1
    ).astype(np.float32)
    return r0, r1, w4


def _prep_core_inputs(x_b, offset_b, mask_b, weight, bias):
    r0, r1, w4 = _prep_indices_weights(offset_b, mask_b)

    x_in = np.ascontiguousarray(
        x_b.reshape(CT, 128, HW).transpose(1, 0, 2)
    ).astype(ml_dtypes.bfloat16)

    wk = weight.reshape(OC, CT, 128, K)
    wr = np.ascontiguousarray(wk.transpose(2, 3, 1, 0)).astype(ml_dtypes.bfloat16)

    rs = np.stack([r0, r1], axis=1).reshape(K, 2, HALVES, NIDX)
    rs = rs.transpose(0, 2, 1, 3)  # [k, half, rsel, i]
    s_idx = np.arange(NIDX // 16)
    p_idx = np.arange(128)
    wrapped = rs[:, :, :, (s_idx[None, :] * 16 + (p_idx[:, None] % 16))]
    idx_in = np.ascontiguousarray(wrapped.transpose(3, 0, 1, 2, 4)).astype(np.int16)

    w4r = w4.reshape(K, 4, JT, 128)
    w4_in = np.ascontiguousarray(w4r.transpose(3, 0, 1, 2)).astype(ml_dtypes.bfloat16)

    bias_in = np.ascontiguousarray(
        np.broadcast_to(bias[None, :], (128, OC))
    ).astype(np.float32)

    return {"x": x_in, "wr": wr, "idx": idx_in, "w4": w4_in, "biasb": bias_in}


def kernel(x, offset, mask, weight, bias):
    x = np.asarray(x, dtype=np.float32)
    offset = np.asarray(offset, dtype=np.float32)
    mask = np.asarray(mask, dtype=np.float32)
    weight = np.asarray(weight, dtype=np.float32)
    bias = np.asarray(bias, dtype=np.float32)

    nc = build_program()
    in_maps = [
        _prep_core_inputs(x[b], offset[b], mask[b], weight, bias)
        for b in range(B)
    ]
    res = run_bass_kernel_spmd(nc, in_maps, core_ids=list(range(B)))

    out = np.empty((B, OC, H, W), dtype=np.float32)
    for b in range(B):
        o = res.results[b]["out"]  # (2, JT, 128, 128)
        out[b] = o.transpose(0, 2, 1, 3).reshape(OC, H, W)
    return out

